# revision 8
# baseline (speedup 1.0000x reference)
"""Trainium2 Bass kernel for ContextualLanguageRefinement (sparse local attention).

Math (per batch b):
  Q = h @ W_Q / sqrt(DS); K = h @ W_K
  scores[t,s] = Q[t].K[s], banded |t-s|<=3, softmax over s
  out = softmax((attn @ h @ W_proj) / tau)  computed as  attn @ (h @ W_proj)

Sharding: data-parallel over batch B=8 across the 8 NeuronCores; the small
weights are replicated (concatenated + pre-scaled + bf16-cast on host).

Per-core device pipeline (fully tiled in 512-token column groups j so DMA,
PE and vector engines pipeline):
  1. h [2048,1024] f32 is DMA-loaded in a 32x32-block-swizzled layout
     (contiguous 128B runs), cast to bf16 on gpsimd, block-transposed on the
     vector engine -> hT tiles [128d, 512t] bf16.
  2. Fused projection YT = Wcat^T @ hT gives Q^T, K^T (padded), hp^T tiles.
  3. Per t-block of 128: banded scores S^T over a 144-wide s-window (two
     lhsT chunks), mask+exp, then one PE matmul against the hp window
     augmented with a ones column yields both the attention-weighted
     combine and the softmax denominator; final 32-way softmax on ACT/DVE.
"""

import numpy as np
import ml_dtypes

import concourse.bass as bass
import concourse.bacc as bacc
import concourse.tile as tile
from concourse import mybir
from concourse.bass_utils import run_bass_kernel_spmd

F32 = mybir.dt.float32
BF16 = mybir.dt.bfloat16

B, T, D = 8, 2048, 1024
DS, KL = 256, 32
SCALE = float(np.sqrt(DS))
MW = 2 * DS + KL          # 544 concatenated projection cols
NJ = T // 512             # 4 column groups
NEG = -1e9

N_CORES = 8


def build_nc():
    nc = bacc.Bacc("TRN2", target_bir_lowering=False, debug=False)

    h_d = nc.dram_tensor("h", [T, D], F32, kind="ExternalInput")
    w_d = nc.dram_tensor("wcat", [D, MW], BF16, kind="ExternalInput")
    o_d = nc.dram_tensor("out", [T, KL], F32, kind="ExternalOutput")

    with tile.TileContext(nc) as tc:
        with (
            tc.tile_pool(name="persist", bufs=1) as pp,
            tc.tile_pool(name="y32", bufs=3) as yp,
            tc.tile_pool(name="yb", bufs=3) as ybp,
            tc.tile_pool(name="blk", bufs=3) as bp,
            tc.tile_pool(name="ppsum", bufs=2, space="PSUM") as ppsum,
            tc.tile_pool(name="tpsum", bufs=1, space="PSUM") as tpsum,
            tc.tile_pool(name="spsum", bufs=2, space="PSUM") as spsum,
            tc.tile_pool(name="lpsum", bufs=2, space="PSUM") as lpsum,
        ):
            # ---------------- persistent tiles ----------------
            wc = pp.tile([128, 8, MW], BF16, tag="wc")
            # hT tiles per (d-chunk c, col group j)
            hbt = [[pp.tile([128, 512], BF16, tag=f"hbt{c}_{j}", name=f"hbt{c}_{j}")
                    for j in range(NJ)] for c in range(8)]
            # Q^T per j: [128, dsc, 512]; K^T padded per j: cols = s-window
            # col g <-> s = 512j - 8 + g; g in [0,640) (zeros outside [8,520))
            qt = [pp.tile([128, 2, 512], BF16, tag=f"qt{j}", name=f"qt{j}") for j in range(NJ)]
            ktp = [pp.tile([128, 2, 640], BF16, tag=f"ktp{j}", name=f"ktp{j}") for j in range(NJ)]
            hpt = [pp.tile([32, 528], BF16, tag=f"hpt{j}", name=f"hpt{j}") for j in range(NJ)]
            mask = pp.tile([128, 256], F32, tag="mask")
            mask0 = pp.tile([128, 256], F32, tag="mask0")
            mask15 = pp.tile([128, 256], F32, tag="mask15")
            ident = pp.tile([32, 32], BF16, tag="ident")

            # masks over the packed score sheet [p, 0:128]=window A, [p,128:256]=B
            # A: s = t0-8+p, t = t0+f  -> band iff p-f in [5,11]
            # B: s = t0+120+p, t = t0+(g-128) -> band iff (g-128)-p in [117,123]
            #    i.e. g-p in [245,251]
            for mk in (mask, mask0, mask15):
                nc.gpsimd.memset(mk[:], 0.0)
                nc.gpsimd.affine_select(
                    out=mk[:, 0:128], in_=mk[:, 0:128],
                    compare_op=mybir.AluOpType.is_ge, fill=NEG,
                    base=-5, channel_multiplier=1, pattern=[[-1, 128]])
                nc.gpsimd.affine_select(
                    out=mk[:, 0:128], in_=mk[:, 0:128],
                    compare_op=mybir.AluOpType.is_ge, fill=NEG,
                    base=11, channel_multiplier=-1, pattern=[[1, 128]])
                nc.gpsimd.affine_select(
                    out=mk[:, 128:256], in_=mk[:, 128:256],
                    compare_op=mybir.AluOpType.is_ge, fill=NEG,
                    base=128 - 245, channel_multiplier=-1, pattern=[[1, 128]])
                nc.gpsimd.affine_select(
                    out=mk[:, 128:256], in_=mk[:, 128:256],
                    compare_op=mybir.AluOpType.is_ge, fill=NEG,
                    base=251 - 128, channel_multiplier=1, pattern=[[-1, 128]])
            # block 0 extra: window-A rows p<8 are s<0 -> invalid
            nc.gpsimd.affine_select(
                out=mask0[:, 0:128], in_=mask0[:, 0:128],
                compare_op=mybir.AluOpType.is_ge, fill=NEG,
                base=-8, channel_multiplier=1, pattern=[[0, 128]])
            # block 15 extra: window-B rows p>7 are s>=2048 -> invalid
            nc.gpsimd.affine_select(
                out=mask15[:, 128:256], in_=mask15[:, 128:256],
                compare_op=mybir.AluOpType.is_ge, fill=NEG,
                base=7, channel_multiplier=-1, pattern=[[0, 128]])

            # identity (bf16) for PE transposes of hp^T windows
            nc.gpsimd.memset(ident[:], 0.0)
            nc.gpsimd.affine_select(
                out=ident[:], in_=ident[:], compare_op=mybir.AluOpType.not_equal,
                fill=1.0, base=0, channel_multiplier=1, pattern=[[-1, 32]])

            # zero the K^T / hp^T padding columns.  [520:640) of every group
            # also guards the window-B matmul's over-read (masked after exp,
            # but must be finite); the [520:528) overlap region of groups
            # j<NJ-1 is overwritten later with real values from group j+1.
            for j in range(NJ):
                nc.vector.memset(ktp[j][:, :, 520:640], 0.0)
                nc.vector.memset(hpt[j][:, 520:528], 0.0)
            nc.vector.memset(ktp[0][:, :, 0:8], 0.0)   # s < 0
            nc.vector.memset(hpt[0][:, 0:8], 0.0)

            # weights: wc[p, c, m] = wcat[128c + p, m]
            nc.sync.dma_start(out=wc[:], in_=bass.AP(
                tensor=w_d[:].tensor, offset=0,
                ap=[[MW, 128], [128 * MW, 8], [1, MW]]))

            def phase1(j):
                # swizzled load + cast + 32x32 block transpose for col group j
                for c in range(8):
                    y32 = yp.tile([128, 16, 32], F32, tag="y32")
                    for q in range(4):
                        in_ap = bass.AP(
                            tensor=h_d[:].tensor,
                            offset=512 * j * D + 128 * c + 32 * q,
                            ap=[[D, 32], [32 * D, 16], [1, 32]])
                        nc.sync.dma_start(out=y32[32 * q:32 * (q + 1)], in_=in_ap)
                    yb = ybp.tile([128, 512], BF16, tag="yb")
                    nc.gpsimd.tensor_copy(
                        out=yb[:], in_=y32[:].rearrange("p k d -> p (k d)"))
                    nc.vector.transpose(out=hbt[c][j][:], in_=yb[:])

            def phase2(j):
                # fused projection for col group j; m order puts K^T first so
                # the next col group's attention unblocks earliest
                for m in (2, 3, 0, 1, 4):
                    moff, mp = (m * 128, 128) if m < 4 else (512, 32)
                    ps = ppsum.tile([mp, 512], F32, tag="pps")
                    for c in range(8):
                        nc.tensor.matmul(
                            ps[:], wc[:, c, moff:moff + mp],
                            hbt[c][j][:],
                            start=(c == 0), stop=(c == 7))
                    eng = nc.vector if (m + j) % 2 == 0 else nc.scalar
                    cp = (lambda o, i: nc.scalar.copy(out=o, in_=i)) \
                        if eng is nc.scalar else \
                        (lambda o, i: nc.vector.tensor_copy(out=o, in_=i))
                    if m < 2:
                        cp(qt[j][:, m, :], ps[:])
                    elif m < 4:
                        cp(ktp[j][:, m - 2, 8:520], ps[:])
                        if j > 0:  # tail overlap of previous group (s>=512j)
                            cp(ktp[j - 1][:, m - 2, 520:528], ps[:, 0:8])
                        if j < NJ - 1:  # head of next group (s in [512j+504, 512j+512))
                            cp(ktp[j + 1][:, m - 2, 0:8], ps[:, 504:512])
                    else:
                        cp(hpt[j][:, 8:520], ps[:, 0:512])
                        if j > 0:
                            cp(hpt[j - 1][:, 520:528], ps[0:32, 0:8])
                        if j < NJ - 1:
                            cp(hpt[j + 1][:, 0:8], ps[0:32, 504:512])

            def phase3(j):
                # attention blocks r=0..3 of col group j (t0 = 512j + 128r)
                for r in range(4):
                    i = 4 * j + r
                    g0 = 128 * r  # window-A start col within ktp[j]/hpt[j]

                    # hp windows + ones column
                    pA = tpsum.tile([128, 32], BF16, tag="pA")
                    nc.tensor.transpose(pA[:], hpt[j][:, g0:g0 + 128], ident[:])
                    hpwA = bp.tile([128, 33], BF16, tag="hpwA")
                    nc.scalar.copy(out=hpwA[:, 0:32], in_=pA[:])
                    nc.vector.memset(hpwA[:, 32:33], 1.0)
                    pB = tpsum.tile([16, 32], BF16, tag="pB")
                    nc.tensor.transpose(
                        pB[:], hpt[j][:, g0 + 128:g0 + 144], ident[:])
                    hpwB = bp.tile([16, 33], BF16, tag="hpwB")
                    nc.scalar.copy(out=hpwB[:, 0:32], in_=pB[:])
                    nc.vector.memset(hpwB[:, 32:33], 1.0)

                    # banded scores S^T, window A in cols 0:128, B in 128:256
                    sps = spsum.tile([128, 256], F32, tag="sps")
                    for dsc in range(2):
                        nc.tensor.matmul(
                            sps[:, 0:128], ktp[j][:, dsc, g0:g0 + 128],
                            qt[j][:, dsc, g0:g0 + 128],
                            start=(dsc == 0), stop=(dsc == 1))
                    for dsc in range(2):
                        nc.tensor.matmul(
                            sps[:, 128:256], ktp[j][:, dsc, g0 + 128:g0 + 256],
                            qt[j][:, dsc, g0:g0 + 128],
                            start=(dsc == 0), stop=(dsc == 1))

                    msk = mask0 if i == 0 else (mask15 if i == 15 else mask)
                    sm = bp.tile([128, 256], F32, tag="sm")
                    nc.vector.tensor_add(out=sm[:], in0=sps[:], in1=msk[:])
                    est = bp.tile([128, 256], BF16, tag="est")
                    nc.scalar.activation(out=est[:], in_=sm[:],
                                         func=mybir.ActivationFunctionType.Exp)

                    # combine: [128t, 33] = sum_s EST[s, t] * [hp | 1][s]
                    lps = lpsum.tile([128, 33], F32, tag="lps")
                    nc.tensor.matmul(lps[:], est[:, 0:128], hpwA[:],
                                     start=True, stop=False)
                    nc.tensor.matmul(lps[:], est[0:16, 128:256], hpwB[:],
                                     start=False, stop=True)

                    # normalize by band sum, then 32-way softmax
                    r1 = bp.tile([128, 1], F32, tag="r1")
                    nc.vector.reciprocal(out=r1[:], in_=lps[:, 32:33])
                    pe = bp.tile([128, KL], F32, tag="pe")
                    se = bp.tile([128, 1], F32, tag="se")
                    nc.scalar.activation(out=pe[:], in_=lps[:, 0:KL],
                                         func=mybir.ActivationFunctionType.Exp,
                                         scale=r1[:], accum_out=se[:])
                    rs = bp.tile([128, 1], F32, tag="rs")
                    nc.vector.reciprocal(out=rs[:], in_=se[:])
                    ot = bp.tile([128, KL], F32, tag="ot")
                    nc.vector.tensor_scalar_mul(out=ot[:], in0=pe[:], scalar1=rs[:])
                    nc.sync.dma_start(
                        out=o_d[128 * i:128 * (i + 1), :], in_=ot[:])

            for j in range(NJ):
                phase1(j)
                phase2(j)
                if j > 0:
                    phase3(j - 1)
            phase3(NJ - 1)

    nc.compile()
    return nc


_NC_CACHE = {}


def _get_nc():
    if "nc" not in _NC_CACHE:
        _NC_CACHE["nc"] = build_nc()
    return _NC_CACHE["nc"]


def kernel(h_base, tau, W_Q, W_K, W_proj):
    h_base = np.asarray(h_base, dtype=np.float32)
    tau_f = float(np.asarray(tau))
    wcat = np.concatenate(
        [np.asarray(W_Q, np.float32) / SCALE,
         np.asarray(W_K, np.float32),
         np.asarray(W_proj, np.float32) / tau_f], axis=1
    ).astype(ml_dtypes.bfloat16)

    nc = _get_nc()
    in_maps = [
        {"h": np.ascontiguousarray(h_base[b]), "wcat": wcat}
        for b in range(B)
    ]
    res = run_bass_kernel_spmd(nc, in_maps, list(range(N_CORES)))
    return np.stack([np.asarray(res.results[b]["out"], dtype=np.float32)
                     for b in range(B)])


# revision 9
# speedup vs baseline: 1.1085x; 1.1085x over previous
"""Trainium2 Bass kernel for ContextualLanguageRefinement (sparse local attention).

Math (per batch b):
  Q = h @ W_Q / sqrt(DS); K = h @ W_K
  scores[t,s] = Q[t].K[s], banded |t-s|<=3, softmax over s
  out = softmax((attn @ h @ W_proj) / tau)  computed as  attn @ (h @ W_proj)

Sharding: data-parallel over batch B=8 across the 8 NeuronCores; the small
weights are replicated (concatenated + pre-scaled + bf16-cast on host).

Per-core device pipeline (fully tiled in 512-token column groups j so DMA,
PE and vector engines pipeline):
  1. h [2048,1024] f32 is DMA-loaded in a 32x32-block-swizzled layout
     (contiguous 128B runs), cast to bf16 on gpsimd, block-transposed on the
     vector engine -> hT tiles [128d, 512t] bf16.
  2. Fused projection YT = Wcat^T @ hT gives Q^T, K^T (padded), hp^T tiles.
  3. Per t-block of 128: banded scores S^T over a 144-wide s-window (two
     lhsT chunks), mask+exp, then one PE matmul against the hp window
     augmented with a ones column yields both the attention-weighted
     combine and the softmax denominator; final 32-way softmax on ACT/DVE.
"""

import numpy as np
import ml_dtypes

import concourse.bass as bass
import concourse.bacc as bacc
import concourse.tile as tile
from concourse import mybir
from concourse.bass_utils import run_bass_kernel_spmd

F32 = mybir.dt.float32
BF16 = mybir.dt.bfloat16

B, T, D = 8, 2048, 1024
DS, KL = 256, 32
SCALE = float(np.sqrt(DS))
MW = 2 * DS + KL          # 544 concatenated projection cols
NJ = T // 512             # 4 column groups
NEG = -1e9

N_CORES = 8


def build_nc():
    nc = bacc.Bacc("TRN2", target_bir_lowering=False, debug=False)

    h_d = nc.dram_tensor("h", [T, D], F32, kind="ExternalInput")
    w_d = nc.dram_tensor("wcat", [D, MW], BF16, kind="ExternalInput")
    o_d = nc.dram_tensor("out", [T, KL], F32, kind="ExternalOutput")

    with tile.TileContext(nc) as tc:
        with (
            tc.tile_pool(name="persist", bufs=1) as pp,
            tc.tile_pool(name="y32", bufs=3) as yp,
            tc.tile_pool(name="yb", bufs=3) as ybp,
            tc.tile_pool(name="blk", bufs=3) as bp,
            tc.tile_pool(name="ppsum", bufs=2, space="PSUM") as ppsum,
            tc.tile_pool(name="tpsum", bufs=1, space="PSUM") as tpsum,
            tc.tile_pool(name="spsum", bufs=2, space="PSUM") as spsum,
            tc.tile_pool(name="lpsum", bufs=2, space="PSUM") as lpsum,
        ):
            # ---------------- persistent tiles ----------------
            wc = pp.tile([128, 8, MW], BF16, tag="wc")
            # hT tiles per (d-chunk c, col group j)
            hbt = [[pp.tile([128, 512], BF16, tag=f"hbt{c}_{j}", name=f"hbt{c}_{j}")
                    for j in range(NJ)] for c in range(8)]
            # Q^T per j: [128, dsc, 512]; K^T padded per j: cols = s-window
            # col g <-> s = 512j - 8 + g; g in [0,640) (zeros outside [8,520))
            qt = [pp.tile([128, 2, 512], BF16, tag=f"qt{j}", name=f"qt{j}") for j in range(NJ)]
            ktp = [pp.tile([128, 2, 640], BF16, tag=f"ktp{j}", name=f"ktp{j}") for j in range(NJ)]
            hpt = [pp.tile([32, 528], BF16, tag=f"hpt{j}", name=f"hpt{j}") for j in range(NJ)]
            mask = pp.tile([128, 256], F32, tag="mask")
            mask0 = pp.tile([128, 256], F32, tag="mask0")
            mask15 = pp.tile([128, 256], F32, tag="mask15")
            ident = pp.tile([32, 32], BF16, tag="ident")

            # masks over the packed score sheet [p, 0:128]=window A, [p,128:256]=B
            # A: s = t0-8+p, t = t0+f  -> band iff p-f in [5,11]
            # B: s = t0+120+p, t = t0+(g-128) -> band iff (g-128)-p in [117,123]
            #    i.e. g-p in [245,251]
            for mk in (mask, mask0, mask15):
                nc.gpsimd.memset(mk[:], 0.0)
                nc.gpsimd.affine_select(
                    out=mk[:, 0:128], in_=mk[:, 0:128],
                    compare_op=mybir.AluOpType.is_ge, fill=NEG,
                    base=-5, channel_multiplier=1, pattern=[[-1, 128]])
                nc.gpsimd.affine_select(
                    out=mk[:, 0:128], in_=mk[:, 0:128],
                    compare_op=mybir.AluOpType.is_ge, fill=NEG,
                    base=11, channel_multiplier=-1, pattern=[[1, 128]])
                nc.gpsimd.affine_select(
                    out=mk[:, 128:256], in_=mk[:, 128:256],
                    compare_op=mybir.AluOpType.is_ge, fill=NEG,
                    base=128 - 245, channel_multiplier=-1, pattern=[[1, 128]])
                nc.gpsimd.affine_select(
                    out=mk[:, 128:256], in_=mk[:, 128:256],
                    compare_op=mybir.AluOpType.is_ge, fill=NEG,
                    base=251 - 128, channel_multiplier=1, pattern=[[-1, 128]])
            # block 0 extra: window-A rows p<8 are s<0 -> invalid
            nc.gpsimd.affine_select(
                out=mask0[:, 0:128], in_=mask0[:, 0:128],
                compare_op=mybir.AluOpType.is_ge, fill=NEG,
                base=-8, channel_multiplier=1, pattern=[[0, 128]])
            # block 15 extra: window-B rows p>7 are s>=2048 -> invalid
            nc.gpsimd.affine_select(
                out=mask15[:, 128:256], in_=mask15[:, 128:256],
                compare_op=mybir.AluOpType.is_ge, fill=NEG,
                base=7, channel_multiplier=-1, pattern=[[0, 128]])

            # identity (bf16) for PE transposes of hp^T windows
            nc.gpsimd.memset(ident[:], 0.0)
            nc.gpsimd.affine_select(
                out=ident[:], in_=ident[:], compare_op=mybir.AluOpType.not_equal,
                fill=1.0, base=0, channel_multiplier=1, pattern=[[-1, 32]])

            # zero the K^T / hp^T padding columns.  [520:640) of every group
            # also guards the window-B matmul's over-read (masked after exp,
            # but must be finite); the [520:528) overlap region of groups
            # j<NJ-1 is overwritten later with real values from group j+1.
            for j in range(NJ):
                nc.vector.memset(ktp[j][:, :, 520:640], 0.0)
                nc.vector.memset(hpt[j][:, 520:528], 0.0)
            nc.vector.memset(ktp[0][:, :, 0:8], 0.0)   # s < 0
            nc.vector.memset(hpt[0][:, 0:8], 0.0)

            # weights: wc[p, c, m] = wcat[128c + p, m]
            nc.sync.dma_start(out=wc[:], in_=bass.AP(
                tensor=w_d[:].tensor, offset=0,
                ap=[[MW, 128], [128 * MW, 8], [1, MW]]))

            def phase1(j):
                # swizzled load + cast + 32x32 block transpose for col group j.
                # The 128B-run pattern is descriptor-heavy, so spread it over
                # all three descriptor generators: SWDGE (gpsimd, casts inline)
                # and both HWDGE rings (SP via nc.sync, ACT via nc.scalar).
                for c in range(8):
                    def in_ap(q):
                        return bass.AP(
                            tensor=h_d[:].tensor,
                            offset=512 * j * D + 128 * c + 32 * q,
                            ap=[[D, 32], [32 * D, 16], [1, 32]])
                    yb = ybp.tile([128, 512], BF16, tag="yb")
                    if c % 2 == 0:
                        # SWDGE path: DMA casts f32->bf16 in the datapath
                        ybv = yb[:].rearrange("p (k d) -> p k d", d=32)
                        for q in range(4):
                            nc.gpsimd.dma_start(
                                out=ybv[32 * q:32 * (q + 1)], in_=in_ap(q))
                    else:
                        y32 = yp.tile([128, 16, 32], F32, tag="y32")
                        eng = nc.sync if c % 4 == 1 else nc.scalar
                        for q in range(4):
                            eng.dma_start(
                                out=y32[32 * q:32 * (q + 1)], in_=in_ap(q))
                        nc.gpsimd.tensor_copy(
                            out=yb[:], in_=y32[:].rearrange("p k d -> p (k d)"))
                    nc.vector.transpose(out=hbt[c][j][:], in_=yb[:])

            def phase2(j):
                # fused projection for col group j; m order puts K^T first so
                # the next col group's attention unblocks earliest
                for m in (2, 3, 0, 1, 4):
                    moff, mp = (m * 128, 128) if m < 4 else (512, 32)
                    ps = ppsum.tile([mp, 512], F32, tag="pps")
                    for c in range(8):
                        nc.tensor.matmul(
                            ps[:], wc[:, c, moff:moff + mp],
                            hbt[c][j][:],
                            start=(c == 0), stop=(c == 7))
                    eng = nc.vector if (m + j) % 2 == 0 else nc.scalar
                    cp = (lambda o, i: nc.scalar.copy(out=o, in_=i)) \
                        if eng is nc.scalar else \
                        (lambda o, i: nc.vector.tensor_copy(out=o, in_=i))
                    if m < 2:
                        cp(qt[j][:, m, :], ps[:])
                    elif m < 4:
                        cp(ktp[j][:, m - 2, 8:520], ps[:])
                        if j > 0:  # tail overlap of previous group (s>=512j)
                            cp(ktp[j - 1][:, m - 2, 520:528], ps[:, 0:8])
                        if j < NJ - 1:  # head of next group (s in [512j+504, 512j+512))
                            cp(ktp[j + 1][:, m - 2, 0:8], ps[:, 504:512])
                    else:
                        cp(hpt[j][:, 8:520], ps[:, 0:512])
                        if j > 0:
                            cp(hpt[j - 1][:, 520:528], ps[0:32, 0:8])
                        if j < NJ - 1:
                            cp(hpt[j + 1][:, 0:8], ps[0:32, 504:512])

            def phase3(j):
                # attention blocks r=0..3 of col group j (t0 = 512j + 128r)
                for r in range(4):
                    i = 4 * j + r
                    g0 = 128 * r  # window-A start col within ktp[j]/hpt[j]

                    # hp windows + ones column
                    pA = tpsum.tile([128, 32], BF16, tag="pA")
                    nc.tensor.transpose(pA[:], hpt[j][:, g0:g0 + 128], ident[:])
                    hpwA = bp.tile([128, 33], BF16, tag="hpwA")
                    nc.scalar.copy(out=hpwA[:, 0:32], in_=pA[:])
                    nc.vector.memset(hpwA[:, 32:33], 1.0)
                    pB = tpsum.tile([16, 32], BF16, tag="pB")
                    nc.tensor.transpose(
                        pB[:], hpt[j][:, g0 + 128:g0 + 144], ident[:])
                    hpwB = bp.tile([16, 33], BF16, tag="hpwB")
                    nc.scalar.copy(out=hpwB[:, 0:32], in_=pB[:])
                    nc.vector.memset(hpwB[:, 32:33], 1.0)

                    # banded scores S^T, window A in cols 0:128, B in 128:256
                    sps = spsum.tile([128, 256], F32, tag="sps")
                    for dsc in range(2):
                        nc.tensor.matmul(
                            sps[:, 0:128], ktp[j][:, dsc, g0:g0 + 128],
                            qt[j][:, dsc, g0:g0 + 128],
                            start=(dsc == 0), stop=(dsc == 1))
                    for dsc in range(2):
                        nc.tensor.matmul(
                            sps[:, 128:256], ktp[j][:, dsc, g0 + 128:g0 + 256],
                            qt[j][:, dsc, g0:g0 + 128],
                            start=(dsc == 0), stop=(dsc == 1))

                    msk = mask0 if i == 0 else (mask15 if i == 15 else mask)
                    sm = bp.tile([128, 256], F32, tag="sm")
                    nc.vector.tensor_add(out=sm[:], in0=sps[:], in1=msk[:])
                    est = bp.tile([128, 256], BF16, tag="est")
                    nc.scalar.activation(out=est[:], in_=sm[:],
                                         func=mybir.ActivationFunctionType.Exp)

                    # combine: [128t, 33] = sum_s EST[s, t] * [hp | 1][s]
                    lps = lpsum.tile([128, 33], F32, tag="lps")
                    nc.tensor.matmul(lps[:], est[:, 0:128], hpwA[:],
                                     start=True, stop=False)
                    nc.tensor.matmul(lps[:], est[0:16, 128:256], hpwB[:],
                                     start=False, stop=True)

                    # normalize by band sum, then 32-way softmax
                    r1 = bp.tile([128, 1], F32, tag="r1")
                    nc.vector.reciprocal(out=r1[:], in_=lps[:, 32:33])
                    pe = bp.tile([128, KL], F32, tag="pe")
                    se = bp.tile([128, 1], F32, tag="se")
                    nc.scalar.activation(out=pe[:], in_=lps[:, 0:KL],
                                         func=mybir.ActivationFunctionType.Exp,
                                         scale=r1[:], accum_out=se[:])
                    rs = bp.tile([128, 1], F32, tag="rs")
                    nc.vector.reciprocal(out=rs[:], in_=se[:])
                    ot = bp.tile([128, KL], F32, tag="ot")
                    nc.vector.tensor_scalar_mul(out=ot[:], in0=pe[:], scalar1=rs[:])
                    nc.sync.dma_start(
                        out=o_d[128 * i:128 * (i + 1), :], in_=ot[:])

            for j in range(NJ):
                phase1(j)
                phase2(j)
                if j > 0:
                    phase3(j - 1)
            phase3(NJ - 1)

    nc.compile()
    return nc


_NC_CACHE = {}


def _get_nc():
    if "nc" not in _NC_CACHE:
        _NC_CACHE["nc"] = build_nc()
    return _NC_CACHE["nc"]


def kernel(h_base, tau, W_Q, W_K, W_proj):
    h_base = np.asarray(h_base, dtype=np.float32)
    tau_f = float(np.asarray(tau))
    wcat = np.concatenate(
        [np.asarray(W_Q, np.float32) / SCALE,
         np.asarray(W_K, np.float32),
         np.asarray(W_proj, np.float32) / tau_f], axis=1
    ).astype(ml_dtypes.bfloat16)

    nc = _get_nc()
    in_maps = [
        {"h": np.ascontiguousarray(h_base[b]), "wcat": wcat}
        for b in range(B)
    ]
    res = run_bass_kernel_spmd(nc, in_maps, list(range(N_CORES)))
    return np.stack([np.asarray(res.results[b]["out"], dtype=np.float32)
                     for b in range(B)])


# revision 10
# speedup vs baseline: 1.1607x; 1.0471x over previous
"""Trainium2 Bass kernel for ContextualLanguageRefinement (sparse local attention).

Math (per batch b):
  Q = h @ W_Q / sqrt(DS); K = h @ W_K
  scores[t,s] = Q[t].K[s], banded |t-s|<=3, softmax over s
  out = softmax((attn @ h @ W_proj) / tau)  computed as  attn @ (h @ W_proj)

Sharding: data-parallel over batch B=8 across the 8 NeuronCores; the small
weights are replicated (concatenated + pre-scaled + bf16-cast on host).

Per-core device pipeline (fully tiled in 512-token column groups j so DMA,
PE and vector engines pipeline):
  1. h [2048,1024] f32 is DMA-loaded in a 32x32-block-swizzled layout
     (contiguous 128B runs), cast to bf16 on gpsimd, block-transposed on the
     vector engine -> hT tiles [128d, 512t] bf16.
  2. Fused projection YT = Wcat^T @ hT gives Q^T, K^T (padded), hp^T tiles.
  3. Per t-block of 128: banded scores S^T over a 144-wide s-window (two
     lhsT chunks), mask+exp, then one PE matmul against the hp window
     augmented with a ones column yields both the attention-weighted
     combine and the softmax denominator; final 32-way softmax on ACT/DVE.
"""

import numpy as np
import ml_dtypes

import concourse.bass as bass
import concourse.bacc as bacc
import concourse.tile as tile
from concourse import mybir
from concourse.bass_utils import run_bass_kernel_spmd

F32 = mybir.dt.float32
BF16 = mybir.dt.bfloat16

B, T, D = 8, 2048, 1024
DS, KL = 256, 32
SCALE = float(np.sqrt(DS))
MW = 2 * DS + KL          # 544 concatenated projection cols
NJ = T // 512             # 4 column groups
NEG = -1e9

N_CORES = 8


def build_nc():
    nc = bacc.Bacc("TRN2", target_bir_lowering=False, debug=False)

    h_d = nc.dram_tensor("h", [T, D], F32, kind="ExternalInput")
    w_d = nc.dram_tensor("wcat", [D, MW], BF16, kind="ExternalInput")
    o_d = nc.dram_tensor("out", [T, KL], F32, kind="ExternalOutput")
    hb_d = nc.dram_tensor("hb_scratch", [T, D], BF16)

    with tile.TileContext(nc) as tc:
        with (
            tc.tile_pool(name="persist", bufs=1) as pp,
            tc.tile_pool(name="y32", bufs=2) as yp,
            tc.tile_pool(name="blk", bufs=3) as bp,
            tc.tile_pool(name="ppsum", bufs=2, space="PSUM") as ppsum,
            tc.tile_pool(name="tpsum", bufs=1, space="PSUM") as tpsum,
            tc.tile_pool(name="spsum", bufs=2, space="PSUM") as spsum,
            tc.tile_pool(name="lpsum", bufs=2, space="PSUM") as lpsum,
        ):
            # ---------------- persistent tiles ----------------
            wc = pp.tile([128, 8, MW], BF16, tag="wc")
            # hT tiles per (d-chunk c, half H=1024 tokens)
            hbt = [[pp.tile([128, 1024], BF16, tag=f"hbt{c}_{hh}", name=f"hbt{c}_{hh}")
                    for hh in range(2)] for c in range(8)]
            # output staging, one DMA at the end
            stg = pp.tile([128, 16, KL], F32, tag="stg")
            # Q^T per j: [128, dsc, 512]; K^T padded per j: cols = s-window
            # col g <-> s = 512j - 8 + g; g in [0,640) (zeros outside [8,520))
            qt = [pp.tile([128, 2, 512], BF16, tag=f"qt{j}", name=f"qt{j}") for j in range(NJ)]
            ktp = [pp.tile([128, 2, 640], BF16, tag=f"ktp{j}", name=f"ktp{j}") for j in range(NJ)]
            hpt = [pp.tile([32, 528], BF16, tag=f"hpt{j}", name=f"hpt{j}") for j in range(NJ)]
            mask = pp.tile([128, 256], F32, tag="mask")
            mask0 = pp.tile([128, 256], F32, tag="mask0")
            mask15 = pp.tile([128, 256], F32, tag="mask15")
            ident = pp.tile([32, 32], BF16, tag="ident")

            # masks over the packed score sheet [p, 0:128]=window A, [p,128:256]=B
            # A: s = t0-8+p, t = t0+f  -> band iff p-f in [5,11]
            # B: s = t0+120+p, t = t0+(g-128) -> band iff (g-128)-p in [117,123]
            #    i.e. g-p in [245,251]
            for mk in (mask, mask0, mask15):
                nc.gpsimd.memset(mk[:], 0.0)
                nc.gpsimd.affine_select(
                    out=mk[:, 0:128], in_=mk[:, 0:128],
                    compare_op=mybir.AluOpType.is_ge, fill=NEG,
                    base=-5, channel_multiplier=1, pattern=[[-1, 128]])
                nc.gpsimd.affine_select(
                    out=mk[:, 0:128], in_=mk[:, 0:128],
                    compare_op=mybir.AluOpType.is_ge, fill=NEG,
                    base=11, channel_multiplier=-1, pattern=[[1, 128]])
                nc.gpsimd.affine_select(
                    out=mk[:, 128:256], in_=mk[:, 128:256],
                    compare_op=mybir.AluOpType.is_ge, fill=NEG,
                    base=128 - 245, channel_multiplier=-1, pattern=[[1, 128]])
                nc.gpsimd.affine_select(
                    out=mk[:, 128:256], in_=mk[:, 128:256],
                    compare_op=mybir.AluOpType.is_ge, fill=NEG,
                    base=251 - 128, channel_multiplier=1, pattern=[[-1, 128]])
            # block 0 extra: window-A rows p<8 are s<0 -> invalid
            nc.gpsimd.affine_select(
                out=mask0[:, 0:128], in_=mask0[:, 0:128],
                compare_op=mybir.AluOpType.is_ge, fill=NEG,
                base=-8, channel_multiplier=1, pattern=[[0, 128]])
            # block 15 extra: window-B rows p>7 are s>=2048 -> invalid
            nc.gpsimd.affine_select(
                out=mask15[:, 128:256], in_=mask15[:, 128:256],
                compare_op=mybir.AluOpType.is_ge, fill=NEG,
                base=7, channel_multiplier=-1, pattern=[[0, 128]])

            # identity (bf16) for PE transposes of hp^T windows
            nc.gpsimd.memset(ident[:], 0.0)
            nc.gpsimd.affine_select(
                out=ident[:], in_=ident[:], compare_op=mybir.AluOpType.not_equal,
                fill=1.0, base=0, channel_multiplier=1, pattern=[[-1, 32]])

            # zero the K^T / hp^T padding columns.  [520:640) of every group
            # also guards the window-B matmul's over-read (masked after exp,
            # but must be finite); the [520:528) overlap region of groups
            # j<NJ-1 is overwritten later with real values from group j+1.
            for j in range(NJ):
                nc.vector.memset(ktp[j][:, :, 520:640], 0.0)
                nc.vector.memset(hpt[j][:, 520:528], 0.0)
            nc.vector.memset(ktp[0][:, :, 0:8], 0.0)   # s < 0
            nc.vector.memset(hpt[0][:, 0:8], 0.0)

            # weights: wc[p, c, m] = wcat[128c + p, m]
            nc.sync.dma_start(out=wc[:], in_=bass.AP(
                tensor=w_d[:].tensor, offset=0,
                ap=[[MW, 128], [128 * MW, 8], [1, MW]]))

            def phase1(hh):
                # 1024 tokens: one big f32 load, one SWDGE cast-store to bf16
                # DRAM scratch, then 8 XBAR DMA-transpose loads -> hT tiles.
                y = yp.tile([128, 8, D], F32, tag="y")
                in_ap = bass.AP(
                    tensor=h_d[:].tensor, offset=1024 * hh * D,
                    ap=[[D, 128], [128 * D, 8], [1, D]])
                nc.sync.dma_start(out=y[:], in_=in_ap)
                out_ap = bass.AP(
                    tensor=hb_d[:].tensor, offset=1024 * hh * D,
                    ap=[[D, 128], [128 * D, 8], [1, D]])
                nc.gpsimd.dma_start(out=out_ap, in_=y[:])  # casts f32->bf16
                for c in range(8):
                    nc.sync.dma_start_transpose(
                        out=hbt[c][hh][:],
                        in_=hb_d[1024 * hh:1024 * (hh + 1), 128 * c:128 * (c + 1)])

            def phase2(j):
                # fused projection for col group j; m order puts K^T first so
                # the next col group's attention unblocks earliest
                for m in (2, 3, 0, 1, 4):
                    moff, mp = (m * 128, 128) if m < 4 else (512, 32)
                    ps = ppsum.tile([mp, 512], F32, tag="pps")
                    for c in range(8):
                        nc.tensor.matmul(
                            ps[:], wc[:, c, moff:moff + mp],
                            hbt[c][j // 2][:, 512 * (j % 2):512 * (j % 2 + 1)],
                            start=(c == 0), stop=(c == 7))
                    eng = nc.vector if (m + j) % 2 == 0 else nc.scalar
                    cp = (lambda o, i: nc.scalar.copy(out=o, in_=i)) \
                        if eng is nc.scalar else \
                        (lambda o, i: nc.vector.tensor_copy(out=o, in_=i))
                    if m < 2:
                        cp(qt[j][:, m, :], ps[:])
                    elif m < 4:
                        cp(ktp[j][:, m - 2, 8:520], ps[:])
                        if j > 0:  # tail overlap of previous group (s>=512j)
                            cp(ktp[j - 1][:, m - 2, 520:528], ps[:, 0:8])
                        if j < NJ - 1:  # head of next group (s in [512j+504, 512j+512))
                            cp(ktp[j + 1][:, m - 2, 0:8], ps[:, 504:512])
                    else:
                        cp(hpt[j][:, 8:520], ps[:, 0:512])
                        if j > 0:
                            cp(hpt[j - 1][:, 520:528], ps[0:32, 0:8])
                        if j < NJ - 1:
                            cp(hpt[j + 1][:, 0:8], ps[0:32, 504:512])

            def phase3(j):
                # attention blocks r=0..3 of col group j (t0 = 512j + 128r)
                for r in range(4):
                    i = 4 * j + r
                    g0 = 128 * r  # window-A start col within ktp[j]/hpt[j]

                    # hp windows + ones column
                    pA = tpsum.tile([128, 32], BF16, tag="pA")
                    nc.tensor.transpose(pA[:], hpt[j][:, g0:g0 + 128], ident[:])
                    hpwA = bp.tile([128, 33], BF16, tag="hpwA")
                    nc.scalar.copy(out=hpwA[:, 0:32], in_=pA[:])
                    nc.vector.memset(hpwA[:, 32:33], 1.0)
                    pB = tpsum.tile([16, 32], BF16, tag="pB")
                    nc.tensor.transpose(
                        pB[:], hpt[j][:, g0 + 128:g0 + 144], ident[:])
                    hpwB = bp.tile([16, 33], BF16, tag="hpwB")
                    nc.scalar.copy(out=hpwB[:, 0:32], in_=pB[:])
                    nc.vector.memset(hpwB[:, 32:33], 1.0)

                    # banded scores S^T, window A in cols 0:128, B in 128:256
                    sps = spsum.tile([128, 256], F32, tag="sps")
                    for dsc in range(2):
                        nc.tensor.matmul(
                            sps[:, 0:128], ktp[j][:, dsc, g0:g0 + 128],
                            qt[j][:, dsc, g0:g0 + 128],
                            start=(dsc == 0), stop=(dsc == 1))
                    for dsc in range(2):
                        nc.tensor.matmul(
                            sps[:, 128:256], ktp[j][:, dsc, g0 + 128:g0 + 256],
                            qt[j][:, dsc, g0:g0 + 128],
                            start=(dsc == 0), stop=(dsc == 1))

                    msk = mask0 if i == 0 else (mask15 if i == 15 else mask)
                    sm = bp.tile([128, 256], F32, tag="sm")
                    nc.vector.tensor_add(out=sm[:], in0=sps[:], in1=msk[:])
                    est = bp.tile([128, 256], BF16, tag="est")
                    nc.scalar.activation(out=est[:], in_=sm[:],
                                         func=mybir.ActivationFunctionType.Exp)

                    # combine: [128t, 33] = sum_s EST[s, t] * [hp | 1][s]
                    lps = lpsum.tile([128, 33], F32, tag="lps")
                    nc.tensor.matmul(lps[:], est[:, 0:128], hpwA[:],
                                     start=True, stop=False)
                    nc.tensor.matmul(lps[:], est[0:16, 128:256], hpwB[:],
                                     start=False, stop=True)

                    # normalize by band sum, then 32-way softmax
                    r1 = bp.tile([128, 1], F32, tag="r1")
                    nc.vector.reciprocal(out=r1[:], in_=lps[:, 32:33])
                    pe = bp.tile([128, KL], F32, tag="pe")
                    se = bp.tile([128, 1], F32, tag="se")
                    nc.scalar.activation(out=pe[:], in_=lps[:, 0:KL],
                                         func=mybir.ActivationFunctionType.Exp,
                                         scale=r1[:], accum_out=se[:])
                    rs = bp.tile([128, 1], F32, tag="rs")
                    nc.vector.reciprocal(out=rs[:], in_=se[:])
                    nc.vector.tensor_scalar_mul(
                        out=stg[:, i, :], in0=pe[:], scalar1=rs[:])

            for hh in range(2):
                phase1(hh)
                for j in (2 * hh, 2 * hh + 1):
                    phase2(j)
                    if j > 0:
                        phase3(j - 1)
            phase3(NJ - 1)
            # single batched output DMA: out[128i + p, k] = stg[p, i, k]
            nc.sync.dma_start(
                out=bass.AP(tensor=o_d[:].tensor, offset=0,
                            ap=[[KL, 128], [128 * KL, 16], [1, KL]]),
                in_=stg[:])

    nc.compile()
    return nc


_NC_CACHE = {}


def _get_nc():
    if "nc" not in _NC_CACHE:
        _NC_CACHE["nc"] = build_nc()
    return _NC_CACHE["nc"]


def kernel(h_base, tau, W_Q, W_K, W_proj):
    h_base = np.asarray(h_base, dtype=np.float32)
    tau_f = float(np.asarray(tau))
    wcat = np.concatenate(
        [np.asarray(W_Q, np.float32) / SCALE,
         np.asarray(W_K, np.float32),
         np.asarray(W_proj, np.float32) / tau_f], axis=1
    ).astype(ml_dtypes.bfloat16)

    nc = _get_nc()
    in_maps = [
        {"h": np.ascontiguousarray(h_base[b]), "wcat": wcat}
        for b in range(B)
    ]
    res = run_bass_kernel_spmd(nc, in_maps, list(range(N_CORES)))
    return np.stack([np.asarray(res.results[b]["out"], dtype=np.float32)
                     for b in range(B)])
